# revision 11
# baseline (speedup 1.0000x reference)
"""Trainium2 Bass kernel for nn_Block_40879498729310 (GPT-style transformer block).

HW-validated: rel err 5.2e-3 (gate 2e-2), 284.6us/iter vs 481.8us baseline
(1.69x).  Design: full-bf16 matmul datapath, DMA-xbar transposes for LN1 (PE
transposes in phase E -- the tile framework serializes xbar transposes with
in-flight collectives), qc-outer attention with per-512-row-slab proj +
pipelined bf16 ReduceScatter issued per chunk, MLP weights prefetched to
SBUF during attention, causal mask applied multiplicatively after exp
(DVE 2x bf16 mode) with the fully non-causal prefix of each diagonal chunk
skipped by exp and zero-filled on the idle Pool engine, softmax denominator
broadcast via a PE ones-matmul (no DRAM round trip), and phase-E MLP
row-chunks interleaved into later
attention iterations behind scheduler-only fences (no_sync_barrier) that
keep RS-dependent waits from head-of-line blocking the engine queues.

Sharding: TP=4 over heads x DP=2 over batch (8 cores).  Core r of a group
computes QKV+attention for heads [3r,3r+3) over its batch's 2048 rows; proj
partials ReduceScatter per 512-row slab; each core then owns the 4 row-chunks
{qc*512 + 128r} and runs LN2+MLP on them locally.
"""
import numpy as np
import ml_dtypes
from contextlib import ExitStack
from functools import lru_cache

import concourse.bass as bass
import concourse.mybir as mybir
import concourse.tile as tile
from concourse.bass_utils import run_bass_kernel_spmd
from concourse.masks import make_identity

F32 = mybir.dt.float32
F32R = mybir.dt.float32r
BF16 = mybir.dt.bfloat16
AF = mybir.ActivationFunctionType
OP = mybir.AluOpType

N_EMBD = 768
N_HEAD = 12
B = 2
T = 2048
HD = 64
GROUP = 4                 # TP group size
HPC = N_HEAD // GROUP     # heads per core = 3
ROWS = T // GROUP         # rows per core in MLP = 512
QC = 512                  # q-chunk width
NQC = T // QC             # 4
EPS = 1e-5
HID = 4 * N_EMBD          # 3072
NHC = HID // 128          # 24 hidden chunks
NDC = N_EMBD // 128       # 6 d chunks

GROUPS = [[0, 1, 2, 3], [4, 5, 6, 7]]


def _split_multi_waits(nc, max_waits=1):
    """Split instructions with >max_waits sem-waits into preceding same-engine
    NoOps (this walrus build rejects multi-wait instructions)."""
    n = 0
    for f in nc.m.functions:
        for bb in f.blocks:
            out = []
            for ins in bb.instructions:
                si = ins.sync_info
                waits = list(si.on_wait) if si is not None else []
                if len(waits) > max_waits:
                    extra, keep = waits[:-max_waits], waits[-max_waits:]
                    for ci in range(0, len(extra), max_waits):
                        nop = mybir.InstNoOp(
                            name=f"{ins.name}-wsplit{ci}",
                            engine=ins.engine,
                            sync_info=mybir.SyncInfo(
                                on_wait=extra[ci:ci + max_waits], on_update=[]),
                            bass_nofuse=True,
                        )
                        out.append(nop)
                        n += 1
                    ins.sync_info = mybir.SyncInfo(
                        on_wait=keep, on_update=list(si.on_update))
                out.append(ins)
            bb.instructions = out
    return n


def build_program(repeat=1):
    nc = bass.Bass(num_devices=8)

    # ---------------- DRAM I/O ----------------
    x_d = nc.declare_dram_parameter("x", [T, N_EMBD], BF16, isOutput=False)
    xown_d = nc.declare_dram_parameter("xown", [4, 128, N_EMBD], BF16, isOutput=False)
    wqk_d = nc.declare_dram_parameter("wqk", [128, NDC, 384], BF16, isOutput=False)
    wv_d = nc.declare_dram_parameter("wv", [128, NDC, HPC * 65], BF16, isOutput=False)
    bqk_d = nc.declare_dram_parameter("bqk", [128, 4], F32, isOutput=False)
    bvb_d = nc.declare_dram_parameter("bvb", [1, HPC * 65], BF16, isOutput=False)
    pw_d = nc.declare_dram_parameter("pw", [HD, HPC, N_EMBD], BF16, isOutput=False)
    fw_d = nc.declare_dram_parameter("fw", [128, NHC, NDC, 128], BF16, isOutput=False)
    fbt_d = nc.declare_dram_parameter("fbt", [128, NHC], F32, isOutput=False)
    f2w_d = nc.declare_dram_parameter("f2w", [128, NHC, N_EMBD], BF16, isOutput=False)
    f2b_d = nc.declare_dram_parameter("f2b", [1, N_EMBD], BF16, isOutput=False)
    masks_d = nc.declare_dram_parameter("masks", [4, 128, QC], BF16, isOutput=False)
    out_d = nc.declare_dram_parameter("out", [4, 128, N_EMBD], F32, isOutput=True)

    proj_slab = [nc.dram_tensor(f"proj_slab{i}", [QC, N_EMBD], BF16)
                 for i in range(NQC)]
    rs_slab = [nc.dram_tensor(f"rs_slab{i}", [128, N_EMBD], BF16)
               for i in range(NQC)]

    def bcast_ap(dram_ap, p):
        return bass.AP(tensor=dram_ap.tensor, offset=dram_ap.offset,
                       ap=[[0, p]] + [list(d) for d in dram_ap.ap])

    with tile.TileContext(nc) as tc, ExitStack() as ctx:
        singles = ctx.enter_context(tc.tile_pool(name="singles", bufs=1))

        eps_t = singles.tile([128, 1], F32, tag="eps")
        nc.vector.memset(eps_t[:], EPS)
        onesf = singles.tile([1, 64], F32, tag="onesf")
        nc.vector.memset(onesf[:], 1.0)
        onesr = singles.tile([1, 64], F32R, tag="onesr")
        nc.vector.tensor_copy(out=onesr[:], in_=onesf[:])
        ident_raw = singles.tile([128, 128], F32, tag="ident_raw")
        make_identity(nc, ident_raw[:])
        identb = singles.tile([128, 128], BF16, tag="identb")
        nc.vector.tensor_copy(out=identb[:], in_=ident_raw[:])

        # critical-path loads first (scalar hwdge queue); the big MLP weight
        # prefetch is emitted after the A/B loop so it queues behind the
        # latency-critical xnT transposes on the scalar queue.
        wqk_sb = singles.tile([128, NDC, 384], BF16, tag="wqk")
        nc.scalar.dma_start(wqk_sb[:], wqk_d[:, :, :])
        wv_sb = singles.tile([128, NDC, HPC * 65], BF16, tag="wv")
        nc.scalar.dma_start(wv_sb[:], wv_d[:, :, :])
        pw_sb = singles.tile([128, HPC, N_EMBD], BF16, tag="pw")
        nc.scalar.dma_start(pw_sb[0:HD, :, :], pw_d[:, :, :])
        bqk_sb = singles.tile([128, 4], F32, tag="bqk")
        nc.gpsimd.dma_start(bqk_sb[:], bqk_d[:, :])
        bvb_sb = singles.tile([128, HPC * 65], BF16, tag="bvb")
        nc.gpsimd.dma_start(bvb_sb[:], bcast_ap(bvb_d[0, :], 128))
        masks_sb = singles.tile([128, 4, QC], BF16, tag="masks")
        f2b_b = singles.tile([128, N_EMBD], BF16, tag="f2b")
        fbt_sb = singles.tile([128, NHC], F32, tag="fbt")
        fw_sb = singles.tile([128, NHC, NDC, 128], BF16, tag="fw")
        f2w_sb = singles.tile([128, NHC, N_EMBD], BF16, tag="f2w")

        for _rep in range(repeat):
          with ExitStack() as s_ctx:
            p_ctx = s_ctx.enter_context(tc.tile_pool(name="p_ctx", bufs=1))
            ctxH = [p_ctx.tile([128, T], BF16, tag=f"ctxH{h}", name=f"ctxH{h}")
                    for h in range(HPC)]
            x2 = [p_ctx.tile([128, N_EMBD], BF16, tag=f"x2_{rt}",
                             name=f"x2_{rt}") for rt in range(4)]
            x2nT = [p_ctx.tile([128, NDC, 256], BF16, tag=f"x2nT{hf}",
                               name=f"x2nT{hf}") for hf in range(2)]
            xo4 = p_ctx.tile([128, 4, N_EMBD], BF16, tag="xo4", name="xo4")
            rst = [p_ctx.tile([128, N_EMBD], BF16, tag=f"rs{rt}",
                              name=f"rs{rt}") for rt in range(4)]
            spool2 = s_ctx.enter_context(tc.tile_pool(name="spool2", bufs=2))
            epool2 = s_ctx.enter_context(tc.tile_pool(name="epool2", bufs=1))
            hpool = s_ctx.enter_context(tc.tile_pool(name="hpool", bufs=1))
            opool = s_ctx.enter_context(tc.tile_pool(name="opool", bufs=1))

            with ExitStack() as q_ctx:
                p_qkv = q_ctx.enter_context(tc.tile_pool(name="p_qkv", bufs=1))
                vaug = [p_qkv.tile([128, HPC * 65], BF16, tag=f"vaug{s}",
                                   name=f"vaug{s}") for s in range(T // 128)]
                qAg, qBg, kAg, kBg = [], [], [], []
                for rg in range(GROUP):
                    for nm, lst in (("qA", qAg), ("qB", qBg), ("kA", kAg),
                                    ("kB", kBg)):
                        lst.append(p_qkv.tile([128, QC], BF16,
                                              tag=f"{nm}{rg}", name=f"{nm}{rg}"))

                psA = q_ctx.enter_context(
                    tc.tile_pool(name="psA", bufs=2, space="PSUM"))
                psS = q_ctx.enter_context(
                    tc.tile_pool(name="psS", bufs=5, space="PSUM"))
                psT = q_ctx.enter_context(
                    tc.tile_pool(name="psT", bufs=1, space="PSUM"))
                spool = q_ctx.enter_context(tc.tile_pool(name="spool", bufs=4))
                epool = q_ctx.enter_context(tc.tile_pool(name="epool", bufs=4))
                rpool = q_ctx.enter_context(tc.tile_pool(name="rpool", bufs=1))
                ppool = q_ctx.enter_context(tc.tile_pool(name="ppool", bufs=1))

                # ---- phase A+B: LN1 -> xnT (dma xbar) -> QKV, per row group ----
                xpool = q_ctx.enter_context(tc.tile_pool(name="xpool", bufs=2))
                xnpool = q_ctx.enter_context(tc.tile_pool(name="xnpool", bufs=4))
                tpool = q_ctx.enter_context(tc.tile_pool(name="tpool", bufs=2))

                def ab_group(rg):
                        xg = xpool.tile([128, 4, N_EMBD], BF16, tag="x_in")
                        nc.sync.dma_start(
                            xg[:],
                            x_d[rg * 512:(rg + 1) * 512, :].rearrange(
                                "(a p) m -> p a m", p=128))
                        xnTg = tpool.tile([128, NDC, QC], BF16, tag="xnT")
                        for rt4 in range(4):
                            xt = xg[:, rt4, :]
                            stats = spool.tile([128, 3, 6], F32, tag="bn_st")
                            xgr = xt.rearrange("p (g c) -> p g c", g=3)
                            for g in range(3):
                                nc.vector.bn_stats(out=stats[:, g, :],
                                                   in_=xgr[:, g, :])
                            mv = spool.tile([128, 2], F32, tag="bn_mv")
                            nc.vector.bn_aggr(out=mv[:], in_=stats[:])
                            sd = spool.tile([128, 2], F32, tag="bn_sd")
                            nc.scalar.activation(sd[:, 0:1], mv[:, 1:2], AF.Sqrt,
                                                 bias=eps_t[:])
                            nc.vector.reciprocal(sd[:, 0:1], sd[:, 0:1])
                            xn = xnpool.tile([128, N_EMBD], BF16, tag="x_n")
                            nc.vector.tensor_scalar(
                                out=xn[:], in0=xt, scalar1=mv[:, 0:1],
                                scalar2=sd[:, 0:1], op0=OP.subtract, op1=OP.mult)
                            nc.scalar.dma_start(
                                xnTg[:, :, rt4 * 128:(rt4 + 1) * 128], xn[:],
                                transpose=True)

                        # QKV^T for q,k (4 sections) + V natural (4 row tiles)
                        secs = [(0, 128, qAg[rg], 0), (128, 64, qBg[rg], 1),
                                (192, 128, kAg[rg], 2), (320, 64, kBg[rg], 3)]
                        for moff, mdim, dst, bidx in secs:
                            ps = psS.tile([128, QC], F32, tag="sc")
                            for dc in range(NDC):
                                nc.tensor.matmul(
                                    ps[:mdim, :], wqk_sb[:, dc, moff:moff + mdim],
                                    xnTg[:, dc, :],
                                    start=(dc == 0), stop=(dc == NDC - 1))
                            nc.vector.tensor_scalar_add(
                                out=dst[:mdim, :], in0=ps[:mdim, :],
                                scalar1=bqk_sb[:mdim, bidx:bidx + 1])
                        for rt4 in range(4):
                            s = rg * 4 + rt4
                            psv = psS.tile([128, QC], F32, tag="sc")
                            for dc in range(NDC):
                                nc.tensor.matmul(
                                    psv[:, 0:HPC * 65],
                                    xnTg[:, dc, rt4 * 128:(rt4 + 1) * 128],
                                    wv_sb[:, dc, :],
                                    start=(dc == 0), stop=(dc == NDC - 1))
                            nc.vector.tensor_tensor(
                                out=vaug[s][:], in0=psv[:, 0:HPC * 65],
                                in1=bvb_sb[:], op=OP.add)

                def e_chunk(rt):
                    """residual + LN2 + FC1 + GELU + FC2 + out for own
                    128-row chunk rt (from slab rt's ReduceScatter)."""
                    rs = rst[rt]
                    nc.vector.tensor_tensor(out=x2[rt][:], in0=rs[:],
                                            in1=xo4[:, rt, :], op=OP.add)
                    stats = spool2.tile([128, 3, 6], F32, tag="bn_st2")
                    xgr = x2[rt][:].rearrange("p (g c) -> p g c", g=3)
                    for g in range(3):
                        nc.vector.bn_stats(out=stats[:, g, :], in_=xgr[:, g, :])
                    mv = spool2.tile([128, 2], F32, tag="bn_mv2")
                    nc.vector.bn_aggr(out=mv[:], in_=stats[:])
                    sd = spool2.tile([128, 2], F32, tag="bn_sd2")
                    nc.scalar.activation(sd[:, 0:1], mv[:, 1:2], AF.Sqrt,
                                         bias=eps_t[:])
                    nc.vector.reciprocal(sd[:, 0:1], sd[:, 0:1])
                    x2n = epool2.tile([128, N_EMBD], BF16, tag="x2n")
                    nc.vector.tensor_scalar(
                        out=x2n[:], in0=x2[rt][:], scalar1=mv[:, 0:1],
                        scalar2=sd[:, 0:1], op0=OP.subtract, op1=OP.mult)
                    xt2 = x2nT[rt // 2]
                    ro = (rt % 2) * 128
                    # PE transpose, not DMA xbar: the tile framework
                    # serializes xbar transposes with in-flight collectives
                    for dc in range(NDC):
                        pt = psT.tile([128, 128], BF16, tag="tp")
                        nc.tensor.transpose(pt[:], x2n[:, dc * 128:(dc + 1) * 128],
                                            identb[:])
                        nc.vector.tensor_copy(out=xt2[:, dc, ro:ro + 128],
                                              in_=pt[:])
                    # FC1 + GELU into hT chunk
                    hT = hpool.tile([128, NHC, 128], BF16, tag="hT")
                    for hc in range(NHC):
                        psf = psS.tile([128, QC], F32, tag="sc")
                        ps = psf[:, 0:128]
                        for dc in range(NDC):
                            nc.tensor.matmul(
                                ps, fw_sb[:, hc, dc, :],
                                xt2[:, dc, ro:ro + 128],
                                start=(dc == 0), stop=(dc == NDC - 1))
                        nc.scalar.activation(
                            hT[:, hc, :], ps, AF.Gelu_apprx_tanh,
                            bias=fbt_sb[:, hc:hc + 1])
                    # FC2 accumulated over all hidden chunks
                    otl = opool.tile([128, N_EMBD], F32, tag="ot")
                    pss = [psS.tile([128, QC], F32, tag="sc",
                                    name=f"fc2ps_{rt}_{ng}")[:, 0:384]
                           for ng in range(2)]
                    for hc in range(NHC):
                        for ng in range(2):
                            nc.tensor.matmul(
                                pss[ng], hT[:, hc, :],
                                f2w_sb[:, hc, ng * 384:(ng + 1) * 384],
                                start=(hc == 0), stop=(hc == NHC - 1))
                    for ng in range(2):
                        gs = slice(ng * 384, (ng + 1) * 384)
                        nc.vector.tensor_tensor(
                            out=otl[:, gs], in0=pss[ng], in1=f2b_b[:, gs],
                            op=OP.add)
                    nc.gpsimd.tensor_tensor(
                        out=otl[:], in0=otl[:], in1=x2[rt][:], op=OP.add)
                    nc.sync.dma_start(out_d[rt, :, :], otl[:])

                # ---- phase C+D: attention per q-chunk, proj + RS pipelined;
                # phase-E row chunks interleave once their RS has landed ----
                def ksl(h, s):
                    t = kAg[s // 4] if h < 2 else kBg[s // 4]
                    po = 64 if h == 1 else 0
                    return t[po:po + 64, (s % 4) * 128:(s % 4 + 1) * 128]

                def qsl(h, qc):
                    t = qAg[qc] if h < 2 else qBg[qc]
                    po = 64 if h == 1 else 0
                    return t[po:po + 64, :]

                for rg in range(GROUP):
                    ab_group(rg)
                # fence so the weight-prefetch transfers queue strictly after
                # the latency-critical AB transposes on the scalar queue
                tc.no_sync_barrier()
                nc.sync.dma_start(
                    xo4[:], xown_d[:, :, :].rearrange("a p m -> p a m"))
                if _rep == 0:
                    nc.gpsimd.dma_start(
                        masks_sb[:], masks_d[:, :, :].rearrange("m p f -> p m f"))
                    nc.gpsimd.dma_start(f2b_b[:], bcast_ap(f2b_d[0, :], 128))
                    nc.gpsimd.dma_start(fbt_sb[:], fbt_d[:, :])
                    for hb in range(NHC // 4):
                        nc.scalar.dma_start(
                            fw_sb[:, hb * 4:(hb + 1) * 4, :, :],
                            fw_d[:, hb * 4:(hb + 1) * 4, :, :])
                    for hb in range(NHC // 4):
                        nc.scalar.dma_start(
                            f2w_sb[:, hb * 4:(hb + 1) * 4, :],
                            f2w_d[:, hb * 4:(hb + 1) * 4, :])

                def att_head(qc, h):
                        pav = psA.tile([128, QC], F32, tag="av")
                        ns = 4 * (qc + 1)
                        ets = {}
                        for s in range(ns):
                            ps = psS.tile([128, QC], F32, tag="sc")
                            nc.tensor.matmul(
                                ps[:], ksl(h, s), qsl(h, qc),
                                start=True, stop=True)
                            et = epool.tile([128, QC], BF16, tag="exp")
                            v = s - (ns - 4)
                            off = max(v, 0) * 128
                            if off:
                                # fully non-causal prefix: zero it (Pool, off
                                # the exp critical path) and exp the rest;
                                # mask v=3 cols 0:384 are all-zero bf16
                                nc.gpsimd.tensor_copy(
                                    out=et[:, 0:off],
                                    in_=masks_sb[:, 3, 0:off])
                            nc.scalar.activation(et[:, off:], ps[:, off:],
                                                 AF.Exp)
                            if v >= 0:
                                # diagonal chunk: mask the boundary region
                                nc.vector.tensor_tensor(
                                    out=et[:, off:], in0=et[:, off:],
                                    in1=masks_sb[:, v, off:],
                                    op=OP.mult)
                            ets[s] = et
                            if s >= 1:
                                nc.tensor.matmul(
                                    pav[0:65, :],
                                    vaug[s - 1][:, h * 65:(h + 1) * 65],
                                    ets.pop(s - 1)[:],
                                    start=(s - 1 == 0), stop=False)
                        nc.tensor.matmul(
                            pav[0:65, :], vaug[ns - 1][:, h * 65:(h + 1) * 65],
                            ets.pop(ns - 1)[:],
                            start=(ns == 1), stop=True)
                        # softmax denominator: row 64 -> SBUF -> recip -> PE bcast
                        rd = rpool.tile([1, QC], F32R, tag="rd")
                        nc.vector.tensor_copy(out=rd[:], in_=pav[64:65, :])
                        with nc.allow_low_precision(reason="f32r is fp32 bits"):
                            nc.vector.reciprocal(rd[:], rd[:])
                        psD = psS.tile([128, QC], F32, tag="sc")
                        nc.tensor.matmul(psD[0:64, :], onesr[:, :], rd[:],
                                         start=True, stop=True)
                        rb = rpool.tile([64, QC], F32, tag="rb")
                        nc.vector.tensor_copy(out=rb[:], in_=psD[0:64, :])
                        nc.vector.tensor_tensor(
                            out=ctxH[h][0:64, qc * QC:(qc + 1) * QC],
                            in0=pav[0:64, :], in1=rb[:], op=OP.mult)

                def do_proj(qc):
                    # proj partial for this q-chunk -> slab -> ReduceScatter
                    pp = ppool.tile([128, 4, N_EMBD], BF16, tag="pp")
                    for rc4 in range(4):
                        rc = qc * 4 + rc4
                        for ng in range(2):
                            psp = psS.tile([128, QC], F32, tag="sc")
                            for h in range(HPC):
                                nc.tensor.matmul(
                                    psp[:, 0:384],
                                    ctxH[h][0:64, rc * 128:(rc + 1) * 128],
                                    pw_sb[0:HD, h, ng * 384:(ng + 1) * 384],
                                    start=(h == 0), stop=(h == HPC - 1))
                            nc.vector.tensor_copy(
                                out=pp[:, rc4, ng * 384:(ng + 1) * 384],
                                in_=psp[:, 0:384])
                    nc.sync.dma_start(
                        proj_slab[qc][:, :].rearrange("(a p) m -> p a m", p=128),
                        pp[:])
                    nc.gpsimd.collective_compute(
                        "ReduceScatter", OP.add,
                        replica_groups=GROUPS,
                        ins=[proj_slab[qc].ap().opt()],
                        outs=[rs_slab[qc].ap().opt()])

                for qc in range(NQC):
                    # scheduler-only fence pins the macro-order: without it
                    # the list scheduler hoists RS-dependent ops ahead, and
                    # their split-wait NoOps head-of-line block engine queues
                    # on the collective semaphore.
                    tc.no_sync_barrier()
                    for h in range(HPC):
                        att_head(qc, h)
                    do_proj(qc)
                    if qc == 3:
                        # RS2 completed before RS3 could start: safe to read
                        # and process slab 2 inside this region
                        nc.sync.dma_start(rst[2][:], rs_slab[2][:, :])
                    if qc >= 2:
                        # emitted after attention: the LN2 chain runs on DVE
                        # under the attention stream, so the FC matmuls land
                        # in the qc-boundary PE bubble with no LN2 stall
                        e_chunk(qc - 2)
                    if qc == 3:
                        e_chunk(2)
                    # fence before the blocking rs-read so the scheduler can't
                    # hoist it into this region and poison DMA-counter waits
                    tc.no_sync_barrier()
                    if qc in (1, 2):
                        # slab qc-1's collective necessarily completed before
                        # RS(qc) could start; this never blocks SP for long
                        nc.sync.dma_start(rst[qc - 1][:], rs_slab[qc - 1][:, :])

                # -------- last phase-E row chunk --------
                nc.sync.dma_start(rst[3][:], rs_slab[3][:, :])
                e_chunk(3)

    _split_multi_waits(nc, max_waits=1)
    return nc


def _host_prep(inputs):
    """Fold LN affines into weights; build per-core input maps."""
    bf = ml_dtypes.bfloat16
    x = np.ascontiguousarray(np.asarray(inputs["x"], dtype=np.float32))
    ln1w = np.asarray(inputs["ln1_w"], np.float32)
    ln1b = np.asarray(inputs["ln1_b"], np.float32)
    aw_full = np.asarray(inputs["attn_w"], np.float32)
    aw = aw_full * ln1w[:, None]
    ab = np.asarray(inputs["attn_b"], np.float32) + ln1b @ aw_full
    aw = aw.copy()
    ab = ab.copy()
    aw[:, :N_EMBD] *= 0.125
    ab[:N_EMBD] *= 0.125
    fw_full = np.asarray(inputs["fc_w"], np.float32)
    fwf = fw_full * np.asarray(inputs["ln2_w"], np.float32)[:, None]
    fbf = np.asarray(inputs["fc_b"], np.float32) + \
        np.asarray(inputs["ln2_b"], np.float32) @ fw_full
    f2w = np.asarray(inputs["fc2_w"], np.float32)
    f2b = np.asarray(inputs["fc2_b"], np.float32)
    pw_full = np.asarray(inputs["proj_w"], np.float32)
    pb = np.asarray(inputs["proj_b"], np.float32)

    # multiplicative 0/1 causal masks for the 4 diagonal 128-row chunks of a
    # 512-col q block (applied to exp(scores): 1=keep, 0=non-causal)
    p = np.arange(128)
    f = np.arange(QC)
    masks = np.ones((4, 128, QC), np.float32)
    for v in range(4):
        masks[v][(p[:, None] + v * 128) > f[None, :]] = 0.0
    masks = masks.astype(bf)

    # fw device layout [128, NHC, NDC, 128]
    fw_dev = np.ascontiguousarray(
        fwf.reshape(NDC, 128, NHC, 128).transpose(1, 2, 0, 3)).astype(bf)
    fbt = np.ascontiguousarray(fbf.reshape(NHC, 128).T)  # [128, NHC]
    f2w_dev = np.ascontiguousarray(
        f2w.reshape(NHC, 128, N_EMBD).transpose(1, 0, 2)).astype(bf)

    in_maps = []
    for core in range(8):
        b = core // GROUP
        r = core % GROUP
        hsl = slice(r * HPC * HD, (r + 1) * HPC * HD)
        wq = aw[:, 0:N_EMBD][:, hsl]          # [768, 192]
        wk = aw[:, N_EMBD:2 * N_EMBD][:, hsl]
        wv = aw[:, 2 * N_EMBD:][:, hsl]
        # q/k transposed-weight slab [128, NDC, 384]: qA|qB|kA|kB
        wqk = np.concatenate([wq, wk], axis=1)  # [768, 384]
        wqk_dev = np.ascontiguousarray(
            wqk.reshape(NDC, 128, 384).transpose(1, 0, 2)).astype(bf)
        bq = ab[0:N_EMBD][hsl]
        bk = ab[N_EMBD:2 * N_EMBD][hsl]
        bv = ab[2 * N_EMBD:][hsl]
        bqk = np.zeros((128, 4), np.float32)
        bqk[:, 0] = bq[0:128]
        bqk[:64, 1] = bq[128:192]
        bqk[:, 2] = bk[0:128]
        bqk[:64, 3] = bk[128:192]
        # V weights augmented with a zero column per head (ones come via bias)
        wv_aug = np.zeros((N_EMBD, HPC * 65), np.float32)
        bvb = np.zeros((1, HPC * 65), np.float32)
        for h in range(HPC):
            wv_aug[:, h * 65:h * 65 + 64] = wv[:, h * 64:(h + 1) * 64]
            bvb[0, h * 65:h * 65 + 64] = bv[h * 64:(h + 1) * 64]
            bvb[0, h * 65 + 64] = 1.0
        wv_dev = np.ascontiguousarray(
            wv_aug.reshape(NDC, 128, HPC * 65).transpose(1, 0, 2)).astype(bf)
        # proj weights [HD, HPC, 768]
        pw_dev = np.ascontiguousarray(
            pw_full[hsl, :].reshape(HPC, HD, N_EMBD).transpose(1, 0, 2)).astype(bf)
        # own rows: {qc*512 + r*128 + i}, with the proj bias pre-added
        xown = np.stack([x[b, qc * 512 + r * 128: qc * 512 + (r + 1) * 128]
                         for qc in range(4)]) + pb[None, None, :]
        in_maps.append({
            "x": x[b].astype(bf),
            "xown": np.ascontiguousarray(xown).astype(bf),
            "wqk": wqk_dev,
            "wv": wv_dev,
            "bqk": bqk,
            "bvb": bvb.astype(bf),
            "pw": pw_dev,
            "fw": fw_dev,
            "fbt": fbt,
            "f2w": f2w_dev,
            "f2b": f2b.reshape(1, -1).astype(bf),
            "masks": masks,
        })
    return in_maps


@lru_cache(maxsize=1)
def _get_program():
    return build_program()


def kernel(**inputs):
    in_maps = _host_prep(inputs)
    nc = _get_program()
    res = run_bass_kernel_spmd(nc, in_maps, list(range(8)))
    out = np.zeros((B, T, N_EMBD), np.float32)
    for core in range(8):
        b, r = core // GROUP, core % GROUP
        for qc in range(4):
            out[b, qc * 512 + r * 128: qc * 512 + (r + 1) * 128] = \
                res.results[core]["out"][qc]
    return out
